# revision 4
# baseline (speedup 1.0000x reference)
"""Trainium2 Bass kernel for nn_BaconAdditionReasoner (segment_reduce).

Math (per row b of 1M):
  a = p1 @ minmax(W1); b = p2 @ minmax(W2)           # [10] each
  s_ij = min(a_i, b_j); one_minus = 1 - clip(s)       # [10,10]
  y_k  = 1 - prod_{i+j=k} one_minus_ij                # 19 anti-diag bins
  y    = y / (sum_k y_k + 1e-9)

Kernel formulation (avoids materializing min/clip and the mask matmul):
  alpha = p1 @ (1 - minmax(W1))  (rows of p1 sum to 1)  -> one_minus rows
  s_log_ij = max(max(ln(alpha_i), ln(1e-6)), ln(beta_j))  [monotone log]
  logP_k = sum over anti-diagonal (stride-9 slices of the flattened 10x10)
  y = (1 - exp(logP)) normalized.

Layout: batch rows on the 128 partitions (R=64 rows per partition per
bigtile, contiguous in HBM per partition). The per-row 10x10 matmuls run
on the PE via 12-row-packed transposes (lhsT = transposed p-block,
rhs = kron(I_12, V)); everything elementwise runs on DVE/ACT.

Sharding: pure data parallel over 8 cores, 131072 rows each.
"""
import sys

if '/opt/trn_rl_repo' not in sys.path:
    sys.path.insert(0, '/opt/trn_rl_repo')

import numpy as np

B = 1048576
N_CORES = 8
RPC = B // N_CORES          # 131072 rows per core
P = 128                     # partitions
R = 64                      # rows per partition per bigtile
BT_ROWS = P * R             # 8192
NT = RPC // BT_ROWS         # 16 bigtiles per core
GROUPS = [12, 12, 12, 12, 12, 4]   # r-slices per PE transpose group (sum=R)
assert sum(GROUPS) == R

CNT = [min(k, 18 - k) + 1 for k in range(19)]
I0 = [max(0, k - 9) for k in range(19)]

_CACHED = {}


def _build_nc():
    import concourse.mybir as mybir
    from concourse.bacc import Bacc
    from concourse.mybir import AluOpType
    from concourse.tile import TileContext

    F32 = mybir.dt.float32
    LOG_LO = float(np.float32(np.log(np.float32(1e-6) + np.float32(1e-12))))

    # Bacc (not Bass): its finalize() runs move_matmul_waits_to_ldweights +
    # generate_event_semaphores, required because walrus allows only one
    # sync wait on a self-loading fp32 Matmult.
    nc = Bacc()
    p1d = nc.dram_tensor("p1", [RPC, 10], F32, kind="ExternalInput")
    p2d = nc.dram_tensor("p2", [RPC, 10], F32, kind="ExternalInput")
    v1d = nc.dram_tensor("v1b", [120, 120], F32, kind="ExternalInput")
    v2d = nc.dram_tensor("v2b", [120, 120], F32, kind="ExternalInput")
    idd = nc.dram_tensor("ident", [128, 128], F32, kind="ExternalInput")
    yd = nc.dram_tensor("y", [RPC, 19], F32, kind="ExternalOutput")

    p1r = p1d[:].rearrange("(t p r) c -> t p (r c)", p=P, r=R)
    p2r = p2d[:].rearrange("(t p r) c -> t p (r c)", p=P, r=R)
    yr = yd[:].rearrange("(t p r) k -> t p (r k)", p=P, r=R)

    with TileContext(nc) as tc:
        with (
            tc.tile_pool(name="const", bufs=1) as cpool,
            tc.tile_pool(name="io", bufs=3) as io,
            tc.tile_pool(name="ab", bufs=2) as abp,
            tc.tile_pool(name="pt", bufs=3) as ptp,
            tc.tile_pool(name="s", bufs=2) as sp,
            tc.tile_pool(name="small", bufs=2) as sm,
            tc.tile_pool(name="tp", bufs=4, space="PSUM") as tpp,
            tc.tile_pool(name="mm", bufs=4, space="PSUM") as mmp,
        ):
            v1t = cpool.tile([120, 120], F32)
            v2t = cpool.tile([120, 120], F32)
            idt = cpool.tile([128, 128], F32)
            nc.sync.dma_start(v1t[:], v1d[:])
            nc.sync.dma_start(v2t[:], v2d[:])
            nc.sync.dma_start(idt[:], idd[:])

            for t in range(NT):
                p1t = io.tile([P, R * 10], F32, tag="p1t")
                p2t = io.tile([P, R * 10], F32, tag="p2t")
                nc.sync.dma_start(p1t[:], p1r[t])
                nc.sync.dma_start(p2t[:], p2r[t])

                abt = abp.tile([P, R, 20], F32, tag="ab")
                r0 = 0
                for gs in GROUPS:
                    K = gs * 10
                    for src, vt, o in ((p1t, v1t, 0), (p2t, v2t, 10)):
                        tp = tpp.tile([K, 128], F32, tag="tp")
                        nc.tensor.transpose(
                            tp[:], src[:, r0 * 10:(r0 + gs) * 10], idt[:])
                        pt = ptp.tile([K, 128], F32, tag="pt")
                        nc.scalar.copy(pt[:], tp[:])
                        mm = mmp.tile([P, K], F32, tag="mm")
                        nc.tensor.matmul(mm[:], pt[:], vt[0:K, 0:K],
                                         start=True, stop=True)
                        nc.scalar.copy(
                            abt[:, r0:r0 + gs, o:o + 10],
                            mm[:].rearrange("p (r c) -> p r c", c=10))
                    r0 += gs

                lab = abp.tile([P, R, 20], F32, tag="lab")
                nc.scalar.activation(
                    lab[:].rearrange("p r c -> p (r c)"),
                    abt[:].rearrange("p r c -> p (r c)"),
                    mybir.ActivationFunctionType.Ln)

                # s_log[:, r, i, j] = max(max(la_i, LOG_LO), lb_j)
                # one stt per i (walrus caps TensorScalarPtr APs at 3 dims)
                st = sp.tile([P, R, 10, 10], F32, tag="s")
                b_v = lab[:, :, 10:20]
                for i in range(10):
                    a_v = lab[:, :, i:i + 1].broadcast_to((P, R, 10))
                    nc.vector.scalar_tensor_tensor(
                        st[:, :, i, :], a_v, LOG_LO, b_v,
                        op0=AluOpType.max, op1=AluOpType.max)

                s_flat = st[:].rearrange("p r a b -> p r (a b)")
                lpt = sm.tile([P, R, 19], F32, tag="lp")
                for k in range(19):
                    cnt = CNT[k]
                    start = 9 * I0[k] + k
                    seg = (s_flat[:, :, start:start + 9 * (cnt - 1) + 1:9]
                           if cnt > 1 else s_flat[:, :, start:start + 1])
                    nc.vector.tensor_reduce(
                        lpt[:, :, k:k + 1], seg,
                        axis=mybir.AxisListType.X, op=AluOpType.add)

                et = sm.tile([P, R, 19], F32, tag="e")
                nc.scalar.activation(
                    et[:].rearrange("p r k -> p (r k)"),
                    lpt[:].rearrange("p r k -> p (r k)"),
                    mybir.ActivationFunctionType.Exp)
                ut = sm.tile([P, R, 19], F32, tag="u")
                nc.vector.tensor_scalar(
                    ut[:].rearrange("p r k -> p (r k)"),
                    et[:].rearrange("p r k -> p (r k)"),
                    -1.0, 1.0, AluOpType.mult, AluOpType.add)
                stt = sm.tile([P, R], F32, tag="S")
                nc.vector.tensor_reduce(stt[:], ut[:],
                                        axis=mybir.AxisListType.X,
                                        op=AluOpType.add)
                nc.vector.tensor_scalar_add(stt[:], stt[:], 1e-9)
                rt = sm.tile([P, R], F32, tag="r")
                nc.vector.reciprocal(rt[:], stt[:])
                yt = sm.tile([P, R, 19], F32, tag="y")
                r_b = rt[:].unsqueeze(2).broadcast_to((P, R, 19))
                nc.vector.tensor_tensor(yt[:], ut[:], r_b, AluOpType.mult)
                nc.sync.dma_start(yr[t], yt[:].rearrange("p r k -> p (r k)"))

    nc.finalize()
    return nc


def _host_consts(W1, W2):
    def mmn(W):
        W = W.astype(np.float32)
        lo = W.min(1, keepdims=True)
        hi = W.max(1, keepdims=True)
        return (W - lo) / (hi - lo + np.float32(1e-8))

    eye12 = np.eye(12, dtype=np.float32)
    v1b = np.kron(eye12, (np.float32(1.0) - mmn(W1))).astype(np.float32)
    v2b = np.kron(eye12, (np.float32(1.0) - mmn(W2))).astype(np.float32)
    ident = np.eye(128, dtype=np.float32)
    return v1b, v2b, ident


def kernel(p1, p2, W1, W2, mask=None, **_unused):
    from concourse.bass_utils import run_bass_kernel_spmd

    if 'nc' not in _CACHED:
        _CACHED['nc'] = _build_nc()
    nc = _CACHED['nc']

    v1b, v2b, ident = _host_consts(W1, W2)
    p1 = np.ascontiguousarray(p1, dtype=np.float32)
    p2 = np.ascontiguousarray(p2, dtype=np.float32)

    in_maps = []
    for c in range(N_CORES):
        sl = slice(c * RPC, (c + 1) * RPC)
        in_maps.append({
            "p1": p1[sl], "p2": p2[sl],
            "v1b": v1b, "v2b": v2b, "ident": ident,
        })
    res = run_bass_kernel_spmd(nc, in_maps, list(range(N_CORES)))
    out = np.concatenate([res.results[c]["y"] for c in range(N_CORES)], axis=0)
    return out.astype(np.float32)


if __name__ == "__main__":
    rng = np.random.default_rng(0)
    p1 = rng.random((B, 10), dtype=np.float32)
    p1 /= p1.sum(1, keepdims=True)
    p2 = rng.random((B, 10), dtype=np.float32)
    p2 /= p2.sum(1, keepdims=True)
    W1 = rng.random((10, 10), dtype=np.float32)
    W2 = rng.random((10, 10), dtype=np.float32)
    y = kernel(p1, p2, W1, W2)
    print("kernel ran, y shape", y.shape, "sum", float(y.sum()))
